# revision 17
# baseline (speedup 1.0000x reference)
"""GNN IntraAgg kernel for Trainium2 — multi-queue dma_gather edition.

The v1 kernel was ~99% bound on GpSimd(Q7) SWDGE descriptor generation:
dma_gather costs ~9ns/index of Q7 software time (plus ~1us fixed/call),
so 200704 indices/core = ~1.8ms serialized on one Q7 core pair.

Fixes here:
1. num_swdge_queues=4 with calls round-robined across queue_num 0-3.
   Each SWDGE queue runs on its own Q7 core pair (cpu_id/2 == queue_num
   in the ucode), so descriptor generation for up to 4 calls proceeds
   concurrently -> ~4x less descgen wall time.
2. 2048-index calls (129 descs/engine, ring holds 1024) to amortize the
   ~1us fixed cost per call.
3. Mask building batched: ONE DVE is_equal per 128-node group builds all
   of that group's column masks at once via a 3D broadcast AP, in bf16
   with LOCAL node ids (0..127 + sentinel -1), instead of ~2000 separate
   [128,128] f32 compares.

Layout recap (unchanged from baseline): rows are split by residue class
s = row % 7 so int16 dma_gather indices (row // 7 < 28572) address a
strided view of the bf16 table (elem_step = 7*128 elements). Per class
the slots of all 49 groups are packed compactly in node order, chunked
into 2048-index calls (tail padded to a full 128-slot column with idx-0
filler + sentinel node ids). The reduce is a per-column mask matmul
accumulated into a [128 nodes, D] PSUM tile; feats_1 = psum/32,
feats_2 = self - feats_1.
"""

import numpy as np

N_EMBED, D = 200000, 128
B, K = 50000, 32
N_CORES = 8
P = 128
B_LOCAL = B // N_CORES            # 6250
G = (B_LOCAL + P - 1) // P        # 49 groups of 128 nodes
B_PAD = G * P                     # 6272
NCLS = 7                          # residue classes
QMAX = (N_EMBED + NCLS - 1) // NCLS  # 28572 rows per class view
EMB_PAD_ROWS = QMAX * NCLS        # 200004
IDX_PER_CALL = 1024               # 65 descs/engine; ring holds 128, and
COLS_PER_CALL = IDX_PER_CALL // P  # 8  the 4-queue rotation spaces same-
#                                       queue calls ~3 rounds apart
NQ = 4                            # SWDGE queues (Q7 core pairs)

_cache: dict = {}


def _plan(ni_pad_all):
    """Common (cross-core) plan: per class, per group, padded slot spans,
    plus the common per-group work list of (class, column) and the nid
    plane layout (one bf16 column per work item, boundary columns are
    emitted once per touching group with that group's local ids)."""
    res = ni_pad_all % NCLS
    cnt = np.zeros((N_CORES, G, NCLS), np.int64)
    for s in range(NCLS):
        cnt[:, :, s] = (res == s).reshape(N_CORES, G, P * K).sum(axis=2)
    ccnt = cnt.max(axis=0)                      # [G, NCLS] common padded count
    plan = {}
    for s in range(NCLS):
        starts = np.concatenate([[0], np.cumsum(ccnt[:, s])])
        total = int(starts[-1])
        ncols = -(-total // P)                  # pad tail to full column
        ncalls = -(-ncols // COLS_PER_CALL)
        plan[s] = dict(starts=starts, total=total, ncols=ncols, ncalls=ncalls)
    # per-group work items (common across cores by construction), grouped
    # into per-class runs: gruns[g] = [(s, c0, c1, pos)] with pos the nid
    # plane column offset of the run; gwork[g] = flat [(s, c)] in run order.
    gwork = []
    gruns = []
    pos = 0
    for g in range(G):
        wg = []
        rg = []
        for s in range(NCLS):
            st = plan[s]["starts"]
            if int(st[g]) == int(st[g + 1]):
                continue
            c0 = int(st[g]) // P
            c1 = min(-(-int(st[g + 1]) // P), plan[s]["ncols"])
            rg.append((s, c0, c1, pos))
            for c in range(c0, c1):
                wg.append((s, c))
            pos += c1 - c0
        gwork.append(wg)
        gruns.append(rg)
    nidw = pos
    return ccnt, plan, gwork, gruns, nidw


def _marshal_core(ni_pad, plan, gwork):
    """Per-core idx plane (wrapped int16) + nid plane (bf16 local ids)."""
    import ml_dtypes

    idx_cols = []
    slot_nid = {}
    for s in range(NCLS):
        ncols = plan[s]["ncols"]
        ids = np.zeros(ncols * P, np.int16)
        nid = np.full(ncols * P, -(10 ** 9), np.int64)
        starts = plan[s]["starts"]
        for g in range(G):
            r = ni_pad[g * P:(g + 1) * P]
            pp, kk = np.nonzero(r % NCLS == s)
            rv = r[pp, kk]
            order = np.argsort(pp, kind="stable")
            pp, rv = pp[order], rv[order]
            o = int(starts[g])
            ids[o:o + len(rv)] = (rv // NCLS).astype(np.int16)
            nid[o:o + len(pp)] = g * P + pp
        slot_nid[s] = nid
        for o in range(0, len(ids), IDX_PER_CALL):
            a = ids[o:o + IDX_PER_CALL]
            w = a.reshape(len(a) // 16, 16).T
            idx_cols.append(np.tile(w, (8, 1)))
    idx_plane = np.concatenate(idx_cols, axis=1)

    nid_cols = []
    for g in range(G):
        for (s, c) in gwork[g]:
            col = slot_nid[s][c * P:(c + 1) * P] - g * P
            colf = np.where((col >= 0) & (col < P), col, -1).astype(np.float32)
            nid_cols.append(colf)
    nid_plane = np.stack(nid_cols, axis=1)  # [P, nidw]
    return (np.ascontiguousarray(idx_plane.astype(np.int16)),
            np.ascontiguousarray(nid_plane.astype(ml_dtypes.bfloat16)))


def build_bass(plan, gwork, gruns, nidw, idxw):
    import concourse.mybir as mybir
    import concourse.tile as tile
    from concourse import bacc
    from concourse.library_config import mlp

    nc = bacc.Bacc(
        "TRN2",
        target_bir_lowering=False,
        debug=False,
        enable_asserts=True,
        num_devices=N_CORES,
        num_swdge_queues=NQ,
    )
    emb = nc.dram_tensor(
        "embedding", [EMB_PAD_ROWS, D], mybir.dt.bfloat16, kind="ExternalInput"
    ).ap()
    sf = nc.dram_tensor(
        "self_feats", [B_PAD, D], mybir.dt.float32, kind="ExternalInput"
    ).ap()
    idxp = nc.dram_tensor(
        "idx_plane", [P, idxw], mybir.dt.int16, kind="ExternalInput"
    ).ap()
    nidp = nc.dram_tensor(
        "nid_plane", [P, nidw], mybir.dt.bfloat16, kind="ExternalInput"
    ).ap()
    iotap = nc.dram_tensor(
        "iota128", [P, P], mybir.dt.bfloat16, kind="ExternalInput"
    ).ap()
    out = nc.dram_tensor(
        "out", [B_PAD, 2 * D], mybir.dt.float32, kind="ExternalOutput"
    ).ap()

    emb_cls = emb.rearrange("(q s) d -> q s d", s=NCLS)

    # idx-plane int16-col offset of each (class, call)
    idx_off = {}
    o = 0
    for s in range(NCLS):
        offs = []
        ncols = plan[s]["ncols"]
        for j in range(plan[s]["ncalls"]):
            cols = min(COLS_PER_CALL, ncols - j * COLS_PER_CALL)
            offs.append((o, cols))
            o += cols * P // 16
        idx_off[s] = offs

    with tile.TileContext(nc) as tc:
        with (
            tc.tile_pool(name="const", bufs=1) as const_tp,
            tc.tile_pool(name="gather", bufs=6) as gather_tp,
            tc.tile_pool(name="mask", bufs=4) as mask_tp,
            tc.tile_pool(name="psum", bufs=4, space="PSUM") as psum_tp,
            tc.tile_pool(name="io", bufs=6) as io_tp,
        ):
            nc.gpsimd.load_library(mlp)
            idx_sb = const_tp.tile([P, idxw], mybir.dt.int16, tag="idx")
            # chunked so the first gather waits ~1us, not a 25KB monolith
            CHUNK = 1024
            for o in range(0, idxw, CHUNK):
                e = min(o + CHUNK, idxw)
                nc.sync.dma_start(out=idx_sb[:, o:e], in_=idxp[:, o:e])
            iota_sb = const_tp.tile([P, P], mybir.dt.bfloat16, tag="iota")
            nc.sync.dma_start(out=iota_sb[:], in_=iotap[:, :])
            nid_sb = const_tp.tile([P, nidw], mybir.dt.bfloat16, tag="nid")
            nc.sync.dma_start(out=nid_sb[:], in_=nidp[:, :])

            tiles = {s: [None] * plan[s]["ncalls"] for s in range(NCLS)}
            issued = [0] * NCLS
            qctr = [0]

            def issue_call(s, j):
                o16, cols = idx_off[s][j]
                gt = gather_tp.tile([P, cols * D], mybir.dt.bfloat16,
                                    tag=f"g{s}")
                nc.gpsimd.dma_gather(
                    out_ap=gt[:].rearrange("p (c d) -> p c d", c=cols, d=D),
                    in_ap=emb_cls[:, s, :],
                    idxs_ap=idx_sb[:, o16: o16 + cols * P // 16],
                    num_idxs=cols * P,
                    num_idxs_reg=cols * P,
                    elem_size=D,
                    elem_step=NCLS * D,
                    queue_num=qctr[0] % NQ,
                )
                qctr[0] += 1
                tiles[s][j] = gt

            def ensure_calls(g):
                for (s, c) in gwork[g]:
                    j = c // COLS_PER_CALL
                    while issued[s] <= j:
                        issue_call(s, issued[s])
                        issued[s] += 1

            masks = [None] * G

            def build_masks(g):
                # one small is_equal per (group, class) run: fine-grained so
                # the first matmuls of the group start after ~1us of DVE
                mg = []
                for (s, c0, c1, pos) in gruns[g]:
                    L = c1 - c0
                    mask = mask_tp.tile([P, L * P], mybir.dt.bfloat16,
                                        tag=f"m{s}")
                    nc.vector.tensor_tensor(
                        out=mask[:].rearrange("p (l n) -> p l n", l=L, n=P),
                        in0=nid_sb[:, pos:pos + L, None].to_broadcast(
                            [P, L, P]),
                        in1=iota_sb[:, None, :].to_broadcast([P, L, P]),
                        op=mybir.AluOpType.is_equal,
                    )
                    mg.append(mask)
                masks[g] = mg

            ensure_calls(0)
            for g0 in range(3):
                build_masks(g0)
            for g in range(G):
                # prefetch gathers a few groups ahead to keep 4 queues fed
                for ga in range(g + 1, min(g + 4, G)):
                    ensure_calls(ga)

                work = gwork[g]
                L = len(work)
                ps = psum_tp.tile([P, D], mybir.dt.float32, tag="ps")
                self_t = io_tp.tile([P, D], mybir.dt.float32, tag="self")
                nc.sync.dma_start(out=self_t[:], in_=sf[g * P:(g + 1) * P, :])
                mi = 0
                for ri, (s, c0, c1, pos) in enumerate(gruns[g]):
                    mask = masks[g][ri]
                    for c in range(c0, c1):
                        j = c // COLS_PER_CALL
                        cl = c % COLS_PER_CALL
                        nc.tensor.matmul(
                            out=ps[:],
                            lhsT=mask[:, (c - c0) * P:(c - c0 + 1) * P],
                            rhs=tiles[s][j][:, cl * D:(cl + 1) * D],
                            start=(mi == 0),
                            stop=(mi == L - 1),
                        )
                        mi += 1

                if g + 3 < G:
                    build_masks(g + 3)

                out_t = io_tp.tile([P, 2 * D], mybir.dt.float32, tag="out")
                nc.scalar.mul(out=out_t[:, :D], in_=ps[:], mul=1.0 / K)
                nc.vector.tensor_tensor(
                    out=out_t[:, D:],
                    in0=self_t[:],
                    in1=out_t[:, :D],
                    op=mybir.AluOpType.subtract,
                )
                nc.sync.dma_start(out=out[g * P:(g + 1) * P, :], in_=out_t[:])

    nc.compile()
    return nc


def kernel(embedding, self_feats, neighbor_idx):
    import ml_dtypes
    from concourse import bass_utils

    embedding = np.asarray(embedding, dtype=np.float32)
    sf = np.asarray(self_feats, dtype=np.float32).reshape(N_CORES, B_LOCAL, D)
    ni = np.asarray(neighbor_idx, dtype=np.int32).reshape(N_CORES, B_LOCAL, K)
    sf_pad = np.zeros((N_CORES, B_PAD, D), np.float32)
    ni_pad = np.zeros((N_CORES, B_PAD, K), np.int32)
    sf_pad[:, :B_LOCAL] = sf
    ni_pad[:, :B_LOCAL] = ni

    ccnt, plan, gwork, gruns, nidw = _plan(ni_pad)
    key = ccnt.tobytes()
    emb_bf = np.zeros((EMB_PAD_ROWS, D), ml_dtypes.bfloat16)
    emb_bf[:N_EMBED] = embedding.astype(ml_dtypes.bfloat16)
    iota = np.tile(
        np.arange(P, dtype=np.float32)[None, :], (P, 1)
    ).astype(ml_dtypes.bfloat16)

    maps = []
    idxw = None
    for c in range(N_CORES):
        idx_plane, nid_plane = _marshal_core(ni_pad[c], plan, gwork)
        idxw = idx_plane.shape[1]
        assert nid_plane.shape[1] == nidw
        maps.append(
            {
                "embedding": emb_bf,
                "self_feats": np.ascontiguousarray(sf_pad[c]),
                "idx_plane": idx_plane,
                "nid_plane": nid_plane,
                "iota128": iota,
            }
        )

    if _cache.get("key") != key:
        _cache["nc"] = build_bass(plan, gwork, gruns, nidw, idxw)
        _cache["key"] = key
    nc = _cache["nc"]
    res = bass_utils.run_bass_kernel_spmd(nc, maps, core_ids=list(range(N_CORES)))
    outs = [res.results[c]["out"][:B_LOCAL] for c in range(N_CORES)]
    return np.concatenate(outs, axis=0)


# revision 19
# speedup vs baseline: 1.0098x; 1.0098x over previous
"""GNN IntraAgg kernel for Trainium2 — multi-queue dma_gather edition.

The v1 kernel was ~99% bound on GpSimd(Q7) SWDGE descriptor generation:
dma_gather costs ~9ns/index of Q7 software time (plus ~1us fixed/call),
so 200704 indices/core = ~1.8ms serialized on one Q7 core pair.

Fixes here:
1. num_swdge_queues=4 with calls round-robined across queue_num 0-3.
   Each SWDGE queue runs on its own Q7 core pair (cpu_id/2 == queue_num
   in the ucode), so descriptor generation for up to 4 calls proceeds
   concurrently -> ~4x less descgen wall time.
2. 2048-index calls (129 descs/engine, ring holds 1024) to amortize the
   ~1us fixed cost per call.
3. Mask building batched: ONE DVE is_equal per 128-node group builds all
   of that group's column masks at once via a 3D broadcast AP, in bf16
   with LOCAL node ids (0..127 + sentinel -1), instead of ~2000 separate
   [128,128] f32 compares.

Layout recap (unchanged from baseline): rows are split by residue class
s = row % 7 so int16 dma_gather indices (row // 7 < 28572) address a
strided view of the bf16 table (elem_step = 7*128 elements). Per class
the slots of all 49 groups are packed compactly in node order, chunked
into 2048-index calls (tail padded to a full 128-slot column with idx-0
filler + sentinel node ids). The reduce is a per-column mask matmul
accumulated into a [128 nodes, D] PSUM tile; feats_1 = psum/32,
feats_2 = self - feats_1.
"""

import numpy as np

N_EMBED, D = 200000, 128
B, K = 50000, 32
N_CORES = 8
P = 128
B_LOCAL = B // N_CORES            # 6250
G = (B_LOCAL + P - 1) // P        # 49 groups of 128 nodes
B_PAD = G * P                     # 6272
NCLS = 7                          # residue classes
QMAX = (N_EMBED + NCLS - 1) // NCLS  # 28572 rows per class view
EMB_PAD_ROWS = QMAX * NCLS        # 200004
IDX_PER_CALL = 1024               # 65 descs/engine; ring holds 128, and
COLS_PER_CALL = IDX_PER_CALL // P  # 8  the 4-queue rotation spaces same-
#                                       queue calls ~3 rounds apart
NQ = 4                            # SWDGE queues (Q7 core pairs)

_cache: dict = {}


def _plan(ni_pad_all):
    """Common (cross-core) plan: per class, per group, padded slot spans,
    plus the common per-group work list of (class, column) and the nid
    plane layout (one bf16 column per work item, boundary columns are
    emitted once per touching group with that group's local ids)."""
    res = ni_pad_all % NCLS
    cnt = np.zeros((N_CORES, G, NCLS), np.int64)
    for s in range(NCLS):
        cnt[:, :, s] = (res == s).reshape(N_CORES, G, P * K).sum(axis=2)
    ccnt = cnt.max(axis=0)                      # [G, NCLS] common padded count
    plan = {}
    for s in range(NCLS):
        starts = np.concatenate([[0], np.cumsum(ccnt[:, s])])
        total = int(starts[-1])
        ncols = -(-total // P)                  # pad tail to full column
        ncalls = -(-ncols // COLS_PER_CALL)
        plan[s] = dict(starts=starts, total=total, ncols=ncols, ncalls=ncalls)
    # per-group work items (common across cores by construction), grouped
    # into per-class runs: gruns[g] = [(s, c0, c1, pos)] with pos the nid
    # plane column offset of the run; gwork[g] = flat [(s, c)] in run order.
    gwork = []
    gruns = []
    pos = 0
    for g in range(G):
        wg = []
        rg = []
        for s in range(NCLS):
            st = plan[s]["starts"]
            if int(st[g]) == int(st[g + 1]):
                continue
            c0 = int(st[g]) // P
            c1 = min(-(-int(st[g + 1]) // P), plan[s]["ncols"])
            rg.append((s, c0, c1, pos))
            for c in range(c0, c1):
                wg.append((s, c))
            pos += c1 - c0
        gwork.append(wg)
        gruns.append(rg)
    nidw = pos
    return ccnt, plan, gwork, gruns, nidw


def _marshal_core(ni_pad, plan, gwork):
    """Per-core idx plane (wrapped int16) + nid plane (bf16 local ids)."""
    import ml_dtypes

    idx_cols = []
    slot_nid = {}
    for s in range(NCLS):
        ncols = plan[s]["ncols"]
        ids = np.zeros(ncols * P, np.int16)
        nid = np.full(ncols * P, -(10 ** 9), np.int64)
        starts = plan[s]["starts"]
        for g in range(G):
            r = ni_pad[g * P:(g + 1) * P]
            pp, kk = np.nonzero(r % NCLS == s)
            rv = r[pp, kk]
            order = np.argsort(pp, kind="stable")
            pp, rv = pp[order], rv[order]
            o = int(starts[g])
            ids[o:o + len(rv)] = (rv // NCLS).astype(np.int16)
            nid[o:o + len(pp)] = g * P + pp
        slot_nid[s] = nid
        for o in range(0, len(ids), IDX_PER_CALL):
            a = ids[o:o + IDX_PER_CALL]
            w = a.reshape(len(a) // 16, 16).T
            idx_cols.append(np.tile(w, (8, 1)))
    idx_plane = np.concatenate(idx_cols, axis=1)

    nid_cols = []
    for g in range(G):
        for (s, c) in gwork[g]:
            col = slot_nid[s][c * P:(c + 1) * P] - g * P
            colf = np.where((col >= 0) & (col < P), col, -1).astype(np.float32)
            nid_cols.append(colf)
    nid_plane = np.stack(nid_cols, axis=1)  # [P, nidw]
    return (np.ascontiguousarray(idx_plane.astype(np.int16)),
            np.ascontiguousarray(nid_plane.astype(ml_dtypes.bfloat16)))


def build_bass(plan, gwork, gruns, nidw, idxw):
    import concourse.mybir as mybir
    import concourse.tile as tile
    from concourse import bacc
    from concourse.library_config import mlp

    nc = bacc.Bacc(
        "TRN2",
        target_bir_lowering=False,
        debug=False,
        enable_asserts=True,
        num_devices=N_CORES,
        num_swdge_queues=NQ,
    )
    emb = nc.dram_tensor(
        "embedding", [EMB_PAD_ROWS, D], mybir.dt.bfloat16, kind="ExternalInput"
    ).ap()
    sf = nc.dram_tensor(
        "self_feats", [B_PAD, D], mybir.dt.float32, kind="ExternalInput"
    ).ap()
    idxp = nc.dram_tensor(
        "idx_plane", [P, idxw], mybir.dt.int16, kind="ExternalInput"
    ).ap()
    nidp = nc.dram_tensor(
        "nid_plane", [P, nidw], mybir.dt.bfloat16, kind="ExternalInput"
    ).ap()
    iotap = nc.dram_tensor(
        "iota128", [P, P], mybir.dt.bfloat16, kind="ExternalInput"
    ).ap()
    out = nc.dram_tensor(
        "out", [B_PAD, 2 * D], mybir.dt.float32, kind="ExternalOutput"
    ).ap()

    emb_cls = emb.rearrange("(q s) d -> q s d", s=NCLS)

    # idx-plane int16-col offset of each (class, call)
    idx_off = {}
    o = 0
    for s in range(NCLS):
        offs = []
        ncols = plan[s]["ncols"]
        for j in range(plan[s]["ncalls"]):
            cols = min(COLS_PER_CALL, ncols - j * COLS_PER_CALL)
            offs.append((o, cols))
            o += cols * P // 16
        idx_off[s] = offs

    with tile.TileContext(nc) as tc:
        with (
            tc.tile_pool(name="const", bufs=1) as const_tp,
            tc.tile_pool(name="gather", bufs=6) as gather_tp,
            tc.tile_pool(name="mask", bufs=4) as mask_tp,
            tc.tile_pool(name="psum", bufs=4, space="PSUM") as psum_tp,
            tc.tile_pool(name="io", bufs=6) as io_tp,
        ):
            nc.gpsimd.load_library(mlp)
            idx_sb = const_tp.tile([P, idxw], mybir.dt.int16, tag="idx")
            nc.sync.dma_start(out=idx_sb[:], in_=idxp[:, :])
            nid_sb = const_tp.tile([P, nidw], mybir.dt.bfloat16, tag="nid")
            nc.sync.dma_start(out=nid_sb[:], in_=nidp[:, :])
            iota_sb = const_tp.tile([P, P], mybir.dt.bfloat16, tag="iota")
            nc.sync.dma_start(out=iota_sb[:], in_=iotap[:, :])

            tiles = {s: [None] * plan[s]["ncalls"] for s in range(NCLS)}
            issued = [0] * NCLS
            qctr = [0]

            def issue_call(s, j):
                o16, cols = idx_off[s][j]
                gt = gather_tp.tile([P, cols * D], mybir.dt.bfloat16,
                                    tag=f"g{s}")
                nc.gpsimd.dma_gather(
                    out_ap=gt[:].rearrange("p (c d) -> p c d", c=cols, d=D),
                    in_ap=emb_cls[:, s, :],
                    idxs_ap=idx_sb[:, o16: o16 + cols * P // 16],
                    num_idxs=cols * P,
                    num_idxs_reg=cols * P,
                    elem_size=D,
                    elem_step=NCLS * D,
                    queue_num=qctr[0] % NQ,
                )
                qctr[0] += 1
                tiles[s][j] = gt

            def ensure_calls(g):
                for (s, c) in gwork[g]:
                    j = c // COLS_PER_CALL
                    while issued[s] <= j:
                        issue_call(s, issued[s])
                        issued[s] += 1

            masks = [None] * G

            def build_masks(g):
                # one small is_equal per (group, class) run: fine-grained so
                # the first matmuls of the group start after ~1us of DVE
                mg = []
                for (s, c0, c1, pos) in gruns[g]:
                    L = c1 - c0
                    mask = mask_tp.tile([P, L * P], mybir.dt.bfloat16,
                                        tag=f"m{s}")
                    nc.vector.tensor_tensor(
                        out=mask[:].rearrange("p (l n) -> p l n", l=L, n=P),
                        in0=nid_sb[:, pos:pos + L, None].to_broadcast(
                            [P, L, P]),
                        in1=iota_sb[:, None, :].to_broadcast([P, L, P]),
                        op=mybir.AluOpType.is_equal,
                    )
                    mg.append(mask)
                masks[g] = mg

            ensure_calls(0)
            for g0 in range(3):
                build_masks(g0)
            for g in range(G):
                # prefetch gathers a few groups ahead to keep 4 queues fed
                for ga in range(g + 1, min(g + 5, G)):
                    ensure_calls(ga)

                work = gwork[g]
                L = len(work)
                ps = psum_tp.tile([P, D], mybir.dt.float32, tag="ps")
                self_t = io_tp.tile([P, D], mybir.dt.float32, tag="self")
                nc.sync.dma_start(out=self_t[:], in_=sf[g * P:(g + 1) * P, :])
                mi = 0
                for ri, (s, c0, c1, pos) in enumerate(gruns[g]):
                    mask = masks[g][ri]
                    for c in range(c0, c1):
                        j = c // COLS_PER_CALL
                        cl = c % COLS_PER_CALL
                        nc.tensor.matmul(
                            out=ps[:],
                            lhsT=mask[:, (c - c0) * P:(c - c0 + 1) * P],
                            rhs=tiles[s][j][:, cl * D:(cl + 1) * D],
                            start=(mi == 0),
                            stop=(mi == L - 1),
                        )
                        mi += 1

                if g + 3 < G:
                    build_masks(g + 3)

                out_t = io_tp.tile([P, 2 * D], mybir.dt.float32, tag="out")
                nc.scalar.mul(out=out_t[:, :D], in_=ps[:], mul=1.0 / K)
                nc.vector.tensor_tensor(
                    out=out_t[:, D:],
                    in0=self_t[:],
                    in1=out_t[:, :D],
                    op=mybir.AluOpType.subtract,
                )
                nc.sync.dma_start(out=out[g * P:(g + 1) * P, :], in_=out_t[:])

    nc.compile()
    return nc


def kernel(embedding, self_feats, neighbor_idx):
    import ml_dtypes
    from concourse import bass_utils

    embedding = np.asarray(embedding, dtype=np.float32)
    sf = np.asarray(self_feats, dtype=np.float32).reshape(N_CORES, B_LOCAL, D)
    ni = np.asarray(neighbor_idx, dtype=np.int32).reshape(N_CORES, B_LOCAL, K)
    sf_pad = np.zeros((N_CORES, B_PAD, D), np.float32)
    ni_pad = np.zeros((N_CORES, B_PAD, K), np.int32)
    sf_pad[:, :B_LOCAL] = sf
    ni_pad[:, :B_LOCAL] = ni

    ccnt, plan, gwork, gruns, nidw = _plan(ni_pad)
    key = ccnt.tobytes()
    emb_bf = np.zeros((EMB_PAD_ROWS, D), ml_dtypes.bfloat16)
    emb_bf[:N_EMBED] = embedding.astype(ml_dtypes.bfloat16)
    iota = np.tile(
        np.arange(P, dtype=np.float32)[None, :], (P, 1)
    ).astype(ml_dtypes.bfloat16)

    maps = []
    idxw = None
    for c in range(N_CORES):
        idx_plane, nid_plane = _marshal_core(ni_pad[c], plan, gwork)
        idxw = idx_plane.shape[1]
        assert nid_plane.shape[1] == nidw
        maps.append(
            {
                "embedding": emb_bf,
                "self_feats": np.ascontiguousarray(sf_pad[c]),
                "idx_plane": idx_plane,
                "nid_plane": nid_plane,
                "iota128": iota,
            }
        )

    if _cache.get("key") != key:
        _cache["nc"] = build_bass(plan, gwork, gruns, nidw, idxw)
        _cache["key"] = key
    nc = _cache["nc"]
    res = bass_utils.run_bass_kernel_spmd(nc, maps, core_ids=list(range(N_CORES)))
    outs = [res.results[c]["out"][:B_LOCAL] for c in range(N_CORES)]
    return np.concatenate(outs, axis=0)


# revision 20
# speedup vs baseline: 1.0794x; 1.0689x over previous
"""GNN IntraAgg kernel for Trainium2 — multi-queue dma_gather edition.

The v1 kernel was ~99% bound on GpSimd(Q7) SWDGE descriptor generation:
dma_gather costs ~9ns/index of Q7 software time (plus ~1us fixed/call),
so 200704 indices/core = ~1.8ms serialized on one Q7 core pair.

Fixes here:
1. num_swdge_queues=4 with calls round-robined across queue_num 0-3.
   Each SWDGE queue runs on its own Q7 core pair (cpu_id/2 == queue_num
   in the ucode), so descriptor generation for up to 4 calls proceeds
   concurrently -> ~4x less descgen wall time.
2. 2048-index calls (129 descs/engine, ring holds 1024) to amortize the
   ~1us fixed cost per call.
3. Mask building batched: ONE DVE is_equal per 128-node group builds all
   of that group's column masks at once via a 3D broadcast AP, in bf16
   with LOCAL node ids (0..127 + sentinel -1), instead of ~2000 separate
   [128,128] f32 compares.

Layout recap (unchanged from baseline): rows are split by residue class
s = row % 7 so int16 dma_gather indices (row // 7 < 28572) address a
strided view of the bf16 table (elem_step = 7*128 elements). Per class
the slots of all 49 groups are packed compactly in node order, chunked
into 2048-index calls (tail padded to a full 128-slot column with idx-0
filler + sentinel node ids). The reduce is a per-column mask matmul
accumulated into a [128 nodes, D] PSUM tile; feats_1 = psum/32,
feats_2 = self - feats_1.
"""

import numpy as np

N_EMBED, D = 200000, 128
B, K = 50000, 32
N_CORES = 8
P = 128
B_LOCAL = B // N_CORES            # 6250
G = (B_LOCAL + P - 1) // P        # 49 groups of 128 nodes
B_PAD = G * P                     # 6272
NCLS = 7                          # residue classes
QMAX = (N_EMBED + NCLS - 1) // NCLS  # 28572 rows per class view
EMB_PAD_ROWS = QMAX * NCLS        # 200004
IDX_PER_CALL = 1024               # 65 descs/engine; ring holds 128, and
COLS_PER_CALL = IDX_PER_CALL // P  # 8  the 4-queue rotation spaces same-
#                                       queue calls ~3 rounds apart
NQ = 4                            # SWDGE queues (Q7 core pairs)

_cache: dict = {}


def _plan(ni_pad_all):
    """Common (cross-core) plan: per class, per group, padded slot spans,
    plus the common per-group work list of (class, column) and the nid
    plane layout (one bf16 column per work item, boundary columns are
    emitted once per touching group with that group's local ids)."""
    res = ni_pad_all % NCLS
    cnt = np.zeros((N_CORES, G, NCLS), np.int64)
    for s in range(NCLS):
        cnt[:, :, s] = (res == s).reshape(N_CORES, G, P * K).sum(axis=2)
    ccnt = cnt.max(axis=0)                      # [G, NCLS] common padded count
    plan = {}
    for s in range(NCLS):
        starts = np.concatenate([[0], np.cumsum(ccnt[:, s])])
        total = int(starts[-1])
        ncols = -(-total // P)                  # pad tail to full column
        ncalls = -(-ncols // COLS_PER_CALL)
        plan[s] = dict(starts=starts, total=total, ncols=ncols, ncalls=ncalls)
    # per-group work items (common across cores by construction), grouped
    # into per-class runs: gruns[g] = [(s, c0, c1, pos)] with pos the nid
    # plane column offset of the run; gwork[g] = flat [(s, c)] in run order.
    gwork = []
    gruns = []
    pos = 0
    for g in range(G):
        wg = []
        rg = []
        for s in range(NCLS):
            st = plan[s]["starts"]
            if int(st[g]) == int(st[g + 1]):
                continue
            c0 = int(st[g]) // P
            c1 = min(-(-int(st[g + 1]) // P), plan[s]["ncols"])
            rg.append((s, c0, c1, pos))
            for c in range(c0, c1):
                wg.append((s, c))
            pos += c1 - c0
        gwork.append(wg)
        gruns.append(rg)
    nidw = pos
    return ccnt, plan, gwork, gruns, nidw


def _marshal_core(ni_pad, plan, gwork):
    """Per-core idx plane (wrapped int16) + nid plane (bf16 local ids)."""
    import ml_dtypes

    idx_cols = []
    slot_nid = {}
    for s in range(NCLS):
        ncols = plan[s]["ncols"]
        ids = np.zeros(ncols * P, np.int16)
        nid = np.full(ncols * P, -(10 ** 9), np.int64)
        starts = plan[s]["starts"]
        for g in range(G):
            r = ni_pad[g * P:(g + 1) * P]
            pp, kk = np.nonzero(r % NCLS == s)
            rv = r[pp, kk]
            order = np.argsort(pp, kind="stable")
            pp, rv = pp[order], rv[order]
            o = int(starts[g])
            ids[o:o + len(rv)] = (rv // NCLS).astype(np.int16)
            nid[o:o + len(pp)] = g * P + pp
        slot_nid[s] = nid
        for o in range(0, len(ids), IDX_PER_CALL):
            a = ids[o:o + IDX_PER_CALL]
            w = a.reshape(len(a) // 16, 16).T
            idx_cols.append(np.tile(w, (8, 1)))
    idx_plane = np.concatenate(idx_cols, axis=1)

    nid_cols = []
    for g in range(G):
        for (s, c) in gwork[g]:
            col = slot_nid[s][c * P:(c + 1) * P] - g * P
            colf = np.where((col >= 0) & (col < P), col, -1).astype(np.float32)
            nid_cols.append(colf)
    nid_plane = np.stack(nid_cols, axis=1)  # [P, nidw]
    return (np.ascontiguousarray(idx_plane.astype(np.int16)),
            np.ascontiguousarray(nid_plane.astype(ml_dtypes.bfloat16)))


def build_bass(plan, gwork, gruns, nidw, idxw):
    import concourse.mybir as mybir
    import concourse.tile as tile
    from concourse import bacc
    from concourse.library_config import mlp

    nc = bacc.Bacc(
        "TRN2",
        target_bir_lowering=False,
        debug=False,
        enable_asserts=True,
        num_devices=N_CORES,
        num_swdge_queues=NQ,
    )
    emb = nc.dram_tensor(
        "embedding", [EMB_PAD_ROWS, D], mybir.dt.bfloat16, kind="ExternalInput"
    ).ap()
    sf = nc.dram_tensor(
        "self_feats", [B_PAD, D], mybir.dt.float32, kind="ExternalInput"
    ).ap()
    idxp = nc.dram_tensor(
        "idx_plane", [P, idxw], mybir.dt.int16, kind="ExternalInput"
    ).ap()
    nidp = nc.dram_tensor(
        "nid_plane", [P, nidw], mybir.dt.bfloat16, kind="ExternalInput"
    ).ap()
    iotap = nc.dram_tensor(
        "iota128", [P, P], mybir.dt.bfloat16, kind="ExternalInput"
    ).ap()
    out = nc.dram_tensor(
        "out", [B_PAD, 2 * D], mybir.dt.float32, kind="ExternalOutput"
    ).ap()

    emb_cls = emb.rearrange("(q s) d -> q s d", s=NCLS)

    # idx-plane int16-col offset of each (class, call)
    idx_off = {}
    o = 0
    for s in range(NCLS):
        offs = []
        ncols = plan[s]["ncols"]
        for j in range(plan[s]["ncalls"]):
            cols = min(COLS_PER_CALL, ncols - j * COLS_PER_CALL)
            offs.append((o, cols))
            o += cols * P // 16
        idx_off[s] = offs

    with tile.TileContext(nc) as tc:
        with (
            tc.tile_pool(name="const", bufs=1) as const_tp,
            tc.tile_pool(name="gather", bufs=6) as gather_tp,
            tc.tile_pool(name="mask", bufs=4) as mask_tp,
            tc.tile_pool(name="psum", bufs=4, space="PSUM") as psum_tp,
            tc.tile_pool(name="io", bufs=6) as io_tp,
        ):
            nc.gpsimd.load_library(mlp)
            idx_sb = const_tp.tile([P, idxw], mybir.dt.int16, tag="idx")
            nc.sync.dma_start(out=idx_sb[:], in_=idxp[:, :])
            nid_sb = const_tp.tile([P, nidw], mybir.dt.bfloat16, tag="nid")
            nc.sync.dma_start(out=nid_sb[:], in_=nidp[:, :])
            iota_sb = const_tp.tile([P, P], mybir.dt.bfloat16, tag="iota")
            nc.sync.dma_start(out=iota_sb[:], in_=iotap[:, :])

            tiles = {s: [None] * plan[s]["ncalls"] for s in range(NCLS)}
            issued = [0] * NCLS
            qctr = [0]

            def issue_call(s, j):
                o16, cols = idx_off[s][j]
                gt = gather_tp.tile([P, cols * D], mybir.dt.bfloat16,
                                    tag=f"g{s}")
                nc.gpsimd.dma_gather(
                    out_ap=gt[:].rearrange("p (c d) -> p c d", c=cols, d=D),
                    in_ap=emb_cls[:, s, :],
                    idxs_ap=idx_sb[:, o16: o16 + cols * P // 16],
                    num_idxs=cols * P,
                    num_idxs_reg=cols * P,
                    elem_size=D,
                    elem_step=NCLS * D,
                    queue_num=qctr[0] % NQ,
                )
                qctr[0] += 1
                tiles[s][j] = gt

            def ensure_calls(g):
                for (s, c) in gwork[g]:
                    j = c // COLS_PER_CALL
                    while issued[s] <= j:
                        issue_call(s, issued[s])
                        issued[s] += 1

            masks = [None] * G

            def build_masks(g):
                # one small is_equal per (group, class) run: fine-grained so
                # the first matmuls of the group start after ~1us of DVE
                mg = []
                for (s, c0, c1, pos) in gruns[g]:
                    L = c1 - c0
                    mask = mask_tp.tile([P, L * P], mybir.dt.bfloat16,
                                        tag=f"m{s}")
                    nc.vector.tensor_tensor(
                        out=mask[:].rearrange("p (l n) -> p l n", l=L, n=P),
                        in0=nid_sb[:, pos:pos + L, None].to_broadcast(
                            [P, L, P]),
                        in1=iota_sb[:, None, :].to_broadcast([P, L, P]),
                        op=mybir.AluOpType.is_equal,
                    )
                    mg.append(mask)
                masks[g] = mg

            ensure_calls(0)
            for g0 in range(3):
                build_masks(g0)
            for g in range(G):
                # prefetch gathers a few groups ahead to keep 4 queues fed
                for ga in range(g + 1, min(g + 5, G)):
                    ensure_calls(ga)

                work = gwork[g]
                L = len(work)
                ps = psum_tp.tile([P, D], mybir.dt.float32, tag="ps")
                self_t = io_tp.tile([P, D], mybir.dt.float32, tag="self")
                nc.sync.dma_start(out=self_t[:], in_=sf[g * P:(g + 1) * P, :])
                mi = 0
                for ri, (s, c0, c1, pos) in enumerate(gruns[g]):
                    mask = masks[g][ri]
                    for c in range(c0, c1):
                        j = c // COLS_PER_CALL
                        cl = c % COLS_PER_CALL
                        nc.tensor.matmul(
                            out=ps[:],
                            lhsT=mask[:, (c - c0) * P:(c - c0 + 1) * P],
                            rhs=tiles[s][j][:, cl * D:(cl + 1) * D],
                            start=(mi == 0),
                            stop=(mi == L - 1),
                        )
                        mi += 1

                if g + 3 < G:
                    build_masks(g + 3)

                out_t = io_tp.tile([P, 2 * D], mybir.dt.float32, tag="out")
                nc.vector.tensor_scalar_mul(
                    out=out_t[:, :D], in0=ps[:], scalar1=1.0 / K
                )
                nc.vector.tensor_tensor(
                    out=out_t[:, D:],
                    in0=self_t[:],
                    in1=out_t[:, :D],
                    op=mybir.AluOpType.subtract,
                )
                nc.sync.dma_start(out=out[g * P:(g + 1) * P, :], in_=out_t[:])

    nc.compile()
    return nc


def kernel(embedding, self_feats, neighbor_idx):
    import ml_dtypes
    from concourse import bass_utils

    embedding = np.asarray(embedding, dtype=np.float32)
    sf = np.asarray(self_feats, dtype=np.float32).reshape(N_CORES, B_LOCAL, D)
    ni = np.asarray(neighbor_idx, dtype=np.int32).reshape(N_CORES, B_LOCAL, K)
    sf_pad = np.zeros((N_CORES, B_PAD, D), np.float32)
    ni_pad = np.zeros((N_CORES, B_PAD, K), np.int32)
    sf_pad[:, :B_LOCAL] = sf
    ni_pad[:, :B_LOCAL] = ni

    ccnt, plan, gwork, gruns, nidw = _plan(ni_pad)
    key = ccnt.tobytes()
    emb_bf = np.zeros((EMB_PAD_ROWS, D), ml_dtypes.bfloat16)
    emb_bf[:N_EMBED] = embedding.astype(ml_dtypes.bfloat16)
    iota = np.tile(
        np.arange(P, dtype=np.float32)[None, :], (P, 1)
    ).astype(ml_dtypes.bfloat16)

    maps = []
    idxw = None
    for c in range(N_CORES):
        idx_plane, nid_plane = _marshal_core(ni_pad[c], plan, gwork)
        idxw = idx_plane.shape[1]
        assert nid_plane.shape[1] == nidw
        maps.append(
            {
                "embedding": emb_bf,
                "self_feats": np.ascontiguousarray(sf_pad[c]),
                "idx_plane": idx_plane,
                "nid_plane": nid_plane,
                "iota128": iota,
            }
        )

    if _cache.get("key") != key:
        _cache["nc"] = build_bass(plan, gwork, gruns, nidw, idxw)
        _cache["key"] = key
    nc = _cache["nc"]
    res = bass_utils.run_bass_kernel_spmd(nc, maps, core_ids=list(range(N_CORES)))
    outs = [res.results[c]["out"][:B_LOCAL] for c in range(N_CORES)]
    return np.concatenate(outs, axis=0)
